# revision 62
# baseline (speedup 1.0000x reference)
"""Trainium2 Bass kernel for nn_Block_15144054685914 (dense transformer block).

Sharding: 8 cores = 2 batch groups (DP) x 4-way tensor parallel.
  core c: batch b = c//4, heads [4*(c%4), 4*(c%4)+4), FFN slice c%4.
One on-device bf16 AllReduce per t-chunk (attention residual) within each
4-core batch group; partial MLP outputs (each incl. 0.25*x1) summed on host.

Math tricks (all exact up to float rounding):
  - alpha softmax + basis mixing folded into Wk/Wv and ve on the HOST
  - rmsnorm(x) scale cancels for Q/K (rmsnorm(rope(c*v)) == rmsnorm(rope(v)))
  - rmsnorm scale for the MLP folds into a per-row s2^2 post-scale
  - softmax 1/sum folds into a post-PV column scale; row sums via ones-matmul
"""

import math
import numpy as np
import ml_dtypes

B, E, H, J = 2, 2048, 16, 4
D = 128
GC = 12
FF = 4 * E
NCORES = 8
HL = H // 4            # local heads per core
HDL = HL * D           # 512
JD = J * D             # 512
FL = FF // 4           # 2048 local ffn rows
EPS = float(np.finfo(np.float32).eps)
T_FULL = 2048
CH = 512               # t-chunk for attention + AllReduce
EC = E // 128          # 16
FCT = FL // 128        # 16 f-tiles

bf16n = ml_dtypes.bfloat16
NOAR = False


def _bf(x):
    return np.ascontiguousarray(np.asarray(x, dtype=np.float32)).astype(bf16n)


def _softmax(a):
    e = np.exp(a - a.max(-1, keepdims=True))
    return e / e.sum(-1, keepdims=True)


def shard_inputs(x, ve, cos, sin, Wq, Wk, Wv, Wo, alpha_k, alpha_v, Wg,
                 Wfc, Wmlp, T=T_FULL):
    x = np.asarray(x, np.float32)[:, :T]
    ve = np.asarray(ve, np.float32)[:, :T]
    cosf = np.asarray(cos, np.float32)[0, :T, 0, :]   # (T, 64)
    sinf = np.asarray(sin, np.float32)[0, :T, 0, :]
    Wq = np.asarray(Wq, np.float32)
    Wo = np.asarray(Wo, np.float32)
    Wg = np.asarray(Wg, np.float32)
    Wfc = np.asarray(Wfc, np.float32)
    Wmlp = np.asarray(Wmlp, np.float32)

    # fold alpha-softmax basis mix into K/V weights and ve (host, fp32)
    w_k = _softmax(np.asarray(alpha_k, np.float32))      # (H, J)
    w_v = _softmax(np.asarray(alpha_v, np.float32))
    Wk_b = np.asarray(Wk, np.float32).reshape(J, D, E)
    Wv_b = np.asarray(Wv, np.float32).reshape(J, D, E)
    Wk_eff = np.einsum('hj,jde->hde', w_k, Wk_b).reshape(H * D, E)
    Wv_eff = np.einsum('hj,jde->hde', w_v, Wv_b).reshape(H * D, E)
    ve_mix = np.einsum('hj,btjd->bthd', w_v,
                       ve.reshape(B, T, J, D)).reshape(B, T, H * D)

    # exact causal lower-triangular 128x128 pattern (s <= t)
    tri = (np.arange(128)[:, None] <= np.arange(128)[None, :]).astype(np.float32)

    in_maps = []
    for c in range(NCORES):
        b = c // 4
        hg = c % 4
        hsl = slice(hg * HDL, (hg + 1) * HDL)      # head-dim slice of E/heads
        fsl = slice(hg * FL, (hg + 1) * FL)        # ffn slice
        m = {
            "xq": _bf(0.25 * x[b]),                            # (T, E)
            "xT": _bf(x[b].T),                                 # (E, T)
            "vem": _bf(ve_mix[b][:, hsl]),                     # (T, HDL)
            "cos2": _bf(np.concatenate([cosf.T, cosf.T], 0)),  # (128, T)
            "sin2": _bf(np.concatenate([sinf.T, -sinf.T], 0)),  # (128, T)
            "p64": _bf(np.eye(128)[:, list(range(64, 128)) + list(range(64))].T),
            "wqT": _bf(Wq[hsl, :].T),                          # (E, HDL)
            "wkT": _bf(Wk_eff[hsl, :].T),                      # (E, HDL)
            "wvT": _bf(Wv_eff[hsl, :].T),                      # (E, HDL)
            "woT": _bf(Wo.T[hsl, :]),                          # (HDL, E)
            "wfcT": _bf(Wfc.T[:, fsl]),                        # (E, FL)
            "wmlpT": _bf(Wmlp.T[fsl, :]),                      # (FL, E)
            "wgT": _bf(Wg[hg * HL:(hg + 1) * HL, :].T),        # (GC, HL)
            "tri": _bf(tri),                                   # (128, 128)
            "onec": _bf(np.ones((128, 1))),
            "oner": _bf(np.ones((1, 128))),
        }
        in_maps.append(m)
    return in_maps


def declare_io(nc, T):
    import concourse.mybir as mybir
    bf = mybir.dt.bfloat16
    io = {}

    def inp(name, shape, dt=bf):
        io[name] = nc.dram_tensor(name, list(shape), dt, kind="ExternalInput").ap()

    inp("xq", (T, E)); inp("xT", (E, T)); inp("vem", (T, HDL))
    inp("cos2", (128, T)); inp("sin2", (128, T)); inp("p64", (128, 128))
    inp("wqT", (E, HDL)); inp("wkT", (E, HDL)); inp("wvT", (E, HDL))
    inp("woT", (HDL, E)); inp("wfcT", (E, FL)); inp("wmlpT", (FL, E))
    inp("wgT", (GC, HL))
    inp("tri", (128, 128))
    inp("onec", (128, 1)); inp("oner", (1, 128))
    io["out"] = nc.dram_tensor("out", [T, E], bf, kind="ExternalOutput").ap()
    return io


def emit(tc, io, T):
    import concourse.mybir as mybir
    from contextlib import ExitStack

    nc = tc.nc
    bf = mybir.dt.bfloat16
    f32 = mybir.dt.float32
    AF = mybir.ActivationFunctionType
    OP = mybir.AluOpType
    nch = T // CH
    TT = T // 128                  # number of 128-row t-tiles
    qk_ln_scale = 1.0 / (128.0 * 1.44)   # mean over D and the 1.2^2 fold
    inv_sqrt_d = 1.0 / math.sqrt(D)

    with ExitStack() as ctx:
        cpool = ctx.enter_context(tc.tile_pool(name="const", bufs=1))
        big = ctx.enter_context(tc.tile_pool(name="big", bufs=1))
        wk = ctx.enter_context(tc.tile_pool(name="wk", bufs=1))
        colp = ctx.enter_context(tc.tile_pool(name="colp", bufs=1))
        psp = ctx.enter_context(tc.tile_pool(name="psp", bufs=1, space="PSUM"))
        dram = ctx.enter_context(tc.tile_pool(name="dram", bufs=2, space="DRAM"))

        # ---------------- chunk-0 stream prefetch (first on the queue;
        # resident weights follow in order of first use) ----------------
        xt_tiles = {}
        xq_tiles = {}

        def load_xt(ci):
            t = big.tile([128, EC, CH], bf, name=f"xt{ci}", tag="xt", bufs=2)
            nc.sync.dma_start(
                t[:], io["xT"].rearrange("(a p) t -> p a t", p=128)
                [:, :, ci * CH:(ci + 1) * CH])
            xt_tiles[ci] = t

        def load_xq(ci, tt):
            rows = slice(ci * CH + tt * 128, ci * CH + (tt + 1) * 128)
            t = wk.tile([128, E], bf, name=f"xq{ci}_{tt}", tag="xq", bufs=4)
            nc.sync.dma_start(t[:], io["xq"][rows, :])
            xq_tiles[(ci, tt)] = t

        load_xt(0)
        for tt in range(4):
            load_xq(0, tt)

        # ---------------- constants ----------------
        onec = cpool.tile([128, 1], bf)
        nc.sync.dma_start(onec[:], io["onec"][:])
        oner = cpool.tile([1, 128], bf)
        nc.sync.dma_start(oner[:], io["oner"][:])
        tri = cpool.tile([128, 128], bf)
        nc.sync.dma_start(tri[:], io["tri"][:])
        p64 = cpool.tile([128, 128], bf)
        nc.sync.dma_start(p64[:], io["p64"][:])
        wgT = cpool.tile([GC, HL], bf)
        nc.sync.dma_start(wgT[:], io["wgT"][:])
        eps_e = cpool.tile([128, 1], f32)
        nc.vector.memset(eps_e[:], EPS)
        eps_qk = cpool.tile([1, 1], f32)
        nc.vector.memset(eps_qk[:], EPS / 1.44)
        eps_q4 = cpool.tile([128, 1], f32)
        nc.vector.memset(eps_q4[:], EPS / 4.0)

        # ------- resident weights (loaded once; ring shared with MLP Wmlp,
        # which recycles these four slots after the attention phase) -------
        wq_r = big.tile([128, EC, HDL], bf, name="wq_r", tag="wres", bufs=4)
        nc.sync.dma_start(wq_r[:], io["wqT"].rearrange("(a p) n -> p a n", p=128))
        wk_r = big.tile([128, EC, HDL], bf, name="wk_r", tag="wres", bufs=4)
        nc.sync.dma_start(wk_r[:], io["wkT"].rearrange("(a p) n -> p a n", p=128))
        wv_r = big.tile([128, EC, HDL], bf, name="wv_r", tag="wres", bufs=4)
        nc.sync.dma_start(wv_r[:], io["wvT"].rearrange("(a p) n -> p a n", p=128))
        wo_r = big.tile([128, HL, E], bf, name="wo_r", tag="wres", bufs=4)
        nc.sync.dma_start(wo_r[:], io["woT"].rearrange("(a p) n -> p a n", p=128))

        kT = big.tile([128, HL, T], bf)           # final K, feature-major
        vtile = big.tile([128, TT, HDL], bf)      # final V, token-major

        # first 4 wfc f-tiles resident (reused by all 4 MLP quarters): gives
        # each quarter ~17us of PE runway while the DGE generates the next
        # x1t transpose + streams the remaining f-tiles.
        wfc_res = []
        for f in range(4):
            t = big.tile([128, EC, 128], bf, name=f"wfcr{f}", tag="wfcr",
                         bufs=4)
            nc.sync.dma_start(
                t[:], io["wfcT"].rearrange("(a p) n -> p a n", p=128)
                [:, :, f * 128:(f + 1) * 128])
            wfc_res.append(t)

        # per-chunk DRAM bounce tiles: separate tensors so MLP reads of
        # quarter q depend only on AR(q), not on every AllReduce (the dep
        # tracker is per-tensor for DRAM).
        cins = [dram.tile([CH, E], bf, name=f"cin{c}", tag=f"cin{c}")
                for c in range(nch)]
        couts = [dram.tile([CH, E], bf, name=f"cout{c}", tag=f"cout{c}")
                 for c in range(nch)]

        scols = []     # per t-tile rmsnorm(x) scale (128,1) f32

        groups = [[0, 1, 2, 3], [4, 5, 6, 7]]

        def row_stats_sq(x_tt, name):
            """mean of squares per row of (128, E) given as 2 half tiles
            -> (128,1) f32."""
            bnt = colp.tile([128, 4, 6], f32, name=f"{name}_bnt", tag="bnt",
                            bufs=2)
            for i in range(4):
                nc.vector.bn_stats(bnt[:, i, :],
                                   x_tt[:, i * 512:(i + 1) * 512])
            agg = colp.tile([128, 2], f32, name=f"{name}_agg", tag="agg",
                            bufs=2)
            nc.vector.bn_aggr(agg[:], bnt[:])
            m2 = colp.tile([128, 1], f32, name=f"{name}_m2", tag="c1", bufs=8)
            nc.vector.tensor_tensor(m2[:], agg[:, 0:1], agg[:, 0:1], op=OP.mult)
            msq = colp.tile([128, 1], f32, name=f"{name}_msq", tag="c1", bufs=8)
            nc.vector.tensor_tensor(msq[:], m2[:], agg[:, 1:2], op=OP.add)
            return msq

        # MLP x1 loaders. x1t halves are transpose-DMAs; each is emitted
        # right after its chunk's AllReduce so it lands in the window
        # between that AR and the next one (transpose DMAs serialize
        # against in-flight collectives).
        x1t_tiles = {}
        x1_tts = {}
        s2sqs = {}

        def load_x1t(qi):
            t = big.tile([128, EC, 512], bf, name=f"x1t{qi}", tag="xt",
                         bufs=2)
            nc.sync.dma_start_transpose(t[:], couts[qi][:, :])
            x1t_tiles[qi] = t

        def load_x1rows(qi):
            for tt in range(4):
                rows = slice(tt * 128, (tt + 1) * 128)
                x1_tt = wk.tile([128, E], bf, name=f"x1{qi}_{tt}", tag="xq",
                                bufs=4)
                nc.sync.dma_start(x1_tt[:], couts[qi][rows, :])
                x1_tts[(qi, tt)] = x1_tt

        def x1_stats(qi):
            for tt in range(4):
                x1_tt = x1_tts[(qi, tt)]
                msq1 = row_stats_sq(x1_tt, f"s2_{qi}_{tt}")
                # ln(0.25*(mean+eps)) -> exp(-.) = 4*s2^2; each core then
                # writes mp*4*s2^2 + x1 and the host scales the sum by 0.25.
                ln1 = colp.tile([128, 1], f32, name=f"ln1{qi}_{tt}", tag="c1",
                                bufs=8)
                nc.scalar.activation(ln1[:], msq1[:], AF.Ln, scale=0.25,
                                     bias=eps_q4[:])
                s2sq = colp.tile([128, 1], f32, name=f"s2sq{qi}_{tt}",
                                 tag="s2col", bufs=8)
                nc.scalar.activation(s2sq[:], ln1[:], AF.Exp, scale=-1.0)
                s2sqs[(qi, tt)] = s2sq

        # ======================= attention phase =======================
        rkcol = cpool.tile([128, HL, TT], f32, name="rkcol")
        ln_isd = cpool.tile([1, 1], f32)
        nc.vector.memset(ln_isd[:], math.log(inv_sqrt_d))

        for c in range(nch):
            csl = slice(c * CH, (c + 1) * CH)
            cos2 = wk.tile([128, CH], bf, name=f"cos2_{c}", tag="cs", bufs=2)
            nc.sync.dma_start(cos2[:], io["cos2"][:, csl])
            sin2 = wk.tile([128, CH], bf, name=f"sin2_{c}", tag="cs", bufs=2)
            nc.sync.dma_start(sin2[:], io["sin2"][:, csl])
            xt = xt_tiles.pop(c)

            # prefetch next chunk's streams right away so the loads overlap
            # this chunk's compute instead of stalling the next chunk
            if c + 1 < nch:
                for tt in range(4):
                    load_xq(c + 1, tt)
                load_xt(c + 1)

            # xq t-tiles + s[t] = rsqrt(mean(x^2)+eps) = exp(-0.5*ln(.))
            xq_tts = []
            for tt in range(4):
                xq_tt = xq_tiles.pop((c, tt))
                xq_tts.append(xq_tt)
                msq = row_stats_sq(xq_tt, f"s{c}_{tt}")
                lnm = colp.tile([128, 1], f32, name=f"lnm{c}_{tt}", tag="c1",
                                bufs=8)
                # mean(x^2) = 16*msq  (xq = x/4)
                nc.scalar.activation(lnm[:], msq[:], AF.Ln, scale=16.0,
                                     bias=eps_e[:])
                scol = colp.tile([128, 1], f32, name=f"scol{c}_{tt}",
                                 tag="scol", bufs=4 * nch)
                nc.scalar.activation(scol[:], lnm[:], AF.Exp, scale=-0.5)
                scols.append(scol)

            # ---- gate (token-major) ----
            g3s = []
            for tt in range(4):
                tsl = slice(tt * 128, (tt + 1) * 128)
                g_ps = psp.tile([128, HL], f32, name=f"gps{c}_{tt}", tag="ps",
                                bufs=8)
                nc.tensor.matmul(g_ps[:], xt[0:GC, 0, tsl], wgT[:],
                                 start=True, stop=True)
                zs = colp.tile([128, HL], f32, name=f"zs{c}_{tt}", tag="g4",
                               bufs=3)
                nc.vector.tensor_scalar(zs[:], g_ps[:], scols[c * 4 + tt][:],
                                        None, op0=OP.mult)
                ge = colp.tile([128, HL], f32, name=f"ge{c}_{tt}", tag="g4",
                               bufs=3)
                nc.scalar.activation(ge[:], zs[:], AF.Exp, scale=-1.0)
                gd = colp.tile([128, HL], f32, name=f"gd{c}_{tt}", tag="g4",
                               bufs=3)
                nc.vector.tensor_scalar(gd[:], ge[:], 1.0, None, op0=OP.add)
                gr = colp.tile([128, HL], f32, name=f"gr{c}_{tt}", tag="g4",
                               bufs=3)
                nc.vector.reciprocal(gr[:], gd[:])
                g3 = colp.tile([128, HL], f32, name=f"g3{c}_{tt}", tag="g3",
                               bufs=4)
                nc.vector.tensor_scalar(g3[:], gr[:], 3.0, None, op0=OP.mult)
                g3s.append(g3)

            # ---- q/k pipeline, batched to keep PE dense ----
            # per (kind, h): raw proj -> [sb copy | square] -> ss (ones-mm)
            # -> ln -> exp; rope via swap-mm + vector; k stays unnormalized
            # (its 1/rms folds into the score-exp per-partition scale).
            sbs, sqs, rs2s, qros = {}, {}, {}, {}

            def proj_qk(kind, h, wr):
                ps = psp.tile([128, CH], f32, name=f"{kind}ps{c}_{h}",
                              tag="ps", bufs=8)
                for e in range(EC):
                    nc.tensor.matmul(ps[:], wr[:, e, h * D:(h + 1) * D],
                                     xt[:, e, :],
                                     start=(e == 0), stop=(e == EC - 1))
                sb = wk.tile([128, CH], bf, name=f"{kind}sb{c}_{h}", tag="b1k",
                             bufs=16)
                nc.vector.tensor_copy(sb[:], ps[:])
                # squares via ACT straight from PSUM (frees DVE->GpSimd chain)
                sq = wk.tile([128, CH], bf, name=f"{kind}sq{c}_{h}", tag="sq",
                             bufs=1)
                nc.scalar.activation(sq[:], ps[:], AF.Square)
                sbs[(kind, h)] = sb
                sqs[(kind, h)] = sq

            def norm_mms(kind, h):
                # ss = column sums of sq (ones-mm); ln; per-kind exp
                ss_ps = psp.tile([1, CH], f32, name=f"{kind}ss{c}_{h}",
                                 tag="ps", bufs=8)
                nc.tensor.matmul(ss_ps[:], onec[:], sqs[(kind, h)][:],
                                 start=True, stop=True)
                lnr = psp.tile([1, CH], f32, name=f"{kind}ln{c}_{h}",
                               tag="ps", bufs=8)
                nc.scalar.activation(lnr[:], ss_ps[:], AF.Ln,
                                     scale=qk_ln_scale, bias=eps_qk[:])
                rs2 = colp.tile([1, CH], bf, name=f"{kind}rs{c}_{h}",
                                tag="r512b", bufs=4)
                if kind == "q":
                    nc.scalar.activation(rs2[:], lnr[:], AF.Exp, scale=-0.5)
                else:
                    nc.scalar.activation(rs2[:], lnr[:], AF.Exp, scale=-0.5,
                                         bias=ln_isd[:])
                rs2s[(kind, h)] = rs2

            def swp_rope(kind, h):
                # rope: ro = sb*cos + (p64@sb)*sin; k writes kT directly
                sb = sbs[(kind, h)]
                swp_ps = psp.tile([128, CH], f32, name=f"{kind}sw{c}_{h}",
                                  tag="ps", bufs=8)
                nc.tensor.matmul(swp_ps[:], p64[:], sb[:], start=True,
                                 stop=True)
                ta = wk.tile([128, CH], bf, name=f"{kind}ta{c}_{h}", tag="tt",
                             bufs=2)
                tb = wk.tile([128, CH], bf, name=f"{kind}tb{c}_{h}", tag="tt",
                             bufs=2)
                nc.vector.tensor_tensor(ta[:], sb[:], cos2[:], op=OP.mult)
                nc.vector.tensor_tensor(tb[:], swp_ps[:], sin2[:], op=OP.mult)
                if kind == "k":
                    nc.vector.tensor_tensor(kT[:, h, csl], ta[:], tb[:],
                                            op=OP.add)
                else:
                    ro = wk.tile([128, CH], bf, name=f"qro{c}_{h}", tag="roq",
                                 bufs=2)
                    nc.vector.tensor_tensor(ro[:], ta[:], tb[:], op=OP.add)
                    qros[h] = ro

            def finish_qk(h):
                # q: broadcast 1/rms down 128 partitions, apply to ro
                # (PE matmul: GpSimd is off-limits — the AllReduce lives on
                # its queue and would stall anything queued behind it)
                rb_ps = psp.tile([128, CH], f32, name=f"qrb{c}_{h}",
                                 tag="ps", bufs=8)
                nc.tensor.matmul(rb_ps[:], oner[:], rs2s[("q", h)][:],
                                 start=True, stop=True)
                qf = wk.tile([128, CH], bf, name=f"qf{c}_{h}", tag="b1k",
                             bufs=16)
                nc.vector.tensor_tensor(qf[:], qros[h][:], rb_ps[:],
                                        op=OP.mult)
                qfs[h] = qf
                # k: rs2k*isd as columns of rkcol via 4 tiny transposes
                rs2k = rs2s[("k", h)]
                tr_ps = psp.tile([128, 4], f32, name=f"ktr{c}_{h}", tag="ps",
                                 bufs=8)
                for m in range(4):
                    nc.tensor.matmul(tr_ps[:, m:m + 1],
                                     rs2k[:, m * 128:(m + 1) * 128],
                                     oner[0:1, 0:1], start=True, stop=True)
                nc.vector.tensor_copy(rkcol[:, h, 4 * c:4 * c + 4], tr_ps[:])

            def load_vet(tt):
                rows = slice(c * CH + tt * 128, c * CH + (tt + 1) * 128)
                t = wk.tile([128, HDL], bf, name=f"vet{c}_{tt}", tag="vet",
                            bufs=2)
                nc.sync.dma_start(t[:], io["vem"][rows, :])
                vets[tt] = t

            def v_proj(tt):
                tsl = slice(tt * 128, (tt + 1) * 128)
                v_ps = psp.tile([128, HDL], f32, name=f"vps{c}_{tt}",
                                tag="ps", bufs=8)
                for e in range(EC):
                    nc.tensor.matmul(v_ps[:], xt[:, e, tsl], wv_r[:, e, :],
                                     start=(e == 0), stop=(e == EC - 1))
                if tt + 1 < 4:
                    load_vet(tt + 1)
                vet = vets.pop(tt)
                gv = wk.tile([128, HDL], bf, name=f"gv{c}_{tt}", tag="gv",
                             bufs=1)
                for h in range(HL):
                    nc.vector.tensor_scalar(
                        gv[:, h * D:(h + 1) * D], vet[:, h * D:(h + 1) * D],
                        g3s[tt][:, h:h + 1], None, op0=OP.mult)
                nc.vector.scalar_tensor_tensor(
                    vtile[:, c * 4 + tt, :], v_ps[:], scols[c * 4 + tt][:],
                    gv[:], op0=OP.mult, op1=OP.add)

            qfs = {}
            vets = {}
            load_vet(0)
            for h in (0, 1):
                proj_qk("q", h, wq_r)
                proj_qk("k", h, wk_r)
            for h in (0, 1):
                norm_mms("q", h)
                norm_mms("k", h)
                swp_rope("q", h)
                swp_rope("k", h)
            for h in (2, 3):
                proj_qk("q", h, wq_r)
                proj_qk("k", h, wk_r)
            finish_qk(0)
            finish_qk(1)
            for h in (2, 3):
                norm_mms("q", h)
                norm_mms("k", h)
                swp_rope("q", h)
                swp_rope("k", h)
            v_proj(0)
            v_proj(1)
            finish_qk(2)
            finish_qk(3)
            v_proj(2)
            v_proj(3)
            qfs = [qfs[h] for h in range(HL)]

            # ---- attention (scores in raw-k units; k-norm in exp scale) ----
            # s-block order: diagonal blocks first (suffix-trimmed), then
            # full-width history blocks; last block carries stop.
            order = [(4 * c + m, m * 128, False) for m in range(4)]
            if c > 0:
                order += [(sb, 0, sb == 4 * c - 1) for sb in range(4 * c)]
                skip_chk = False
            else:
                order[-1] = (3, 384, True)
                skip_chk = True      # suffix stop: sim group-check off

            def finish_head(h, sums_ps, yT_ps):
                # 1/sum = exp(-ln(sum)) on ACT (fast; DVE reciprocal on a
                # 1-partition row is ~3us). ln kept in f32: its magnitude
                # is ~20 and bf16 rounding there would cost ~5% error.
                lns = psp.tile([1, CH], f32, name=f"lns{c}_{h}",
                               tag="ps", bufs=8)
                nc.scalar.activation(lns[:], sums_ps[:], AF.Ln)
                isr = colp.tile([1, CH], bf, name=f"isr{c}_{h}", tag="r512b",
                                bufs=4)
                with nc.allow_low_precision(reason="softmax 1/sum in bf16"):
                    nc.scalar.activation(isr[:], lns[:], AF.Exp, scale=-1.0)
                ib_ps = psp.tile([128, CH], f32, name=f"ib{c}_{h}", tag="ps",
                                 bufs=8)
                nc.tensor.matmul(ib_ps[:], oner[:], isr[:], start=True,
                                 stop=True)
                ib = wk.tile([128, CH], bf, name=f"ibs{c}_{h}", tag="p",
                             bufs=2)
                nc.vector.tensor_copy(ib[:], ib_ps[:])
                yTf = wk.tile([128, CH], bf, name=f"yTf{c}_{h}", tag="b1k",
                              bufs=16)
                nc.vector.tensor_tensor(yTf[:], yT_ps[:], ib[:],
                                        op=OP.mult)
                yTfs.append(yTf)

            yTfs = []
            pending = None
            LAG = 3
            for h in range(HL):
                sums_ps = psp.tile([1, CH], f32, name=f"sums{c}_{h}", tag="ps",
                                   bufs=8)
                yT_ps = psp.tile([128, CH], f32, name=f"yT{c}_{h}", tag="ps",
                                 bufs=8)
                pms = {}

                def score_block(bi):
                    sb_i, toff, _ = order[bi]
                    sc_ps = psp.tile([128, CH], f32, name=f"sc{c}_{h}_{sb_i}",
                                     tag="ps", bufs=8)
                    nc.tensor.matmul(sc_ps[:, toff:],
                                     kT[:, h, sb_i * 128:(sb_i + 1) * 128],
                                     qfs[h][:, toff:], start=True, stop=True)
                    pm = wk.tile([128, CH], bf, name=f"pm{c}_{h}_{sb_i}",
                                 tag="pm", bufs=4)
                    nc.scalar.activation(pm[:, toff:], sc_ps[:, toff:],
                                         AF.Exp,
                                         scale=rkcol[:, h, sb_i:sb_i + 1])
                    if sb_i >= 4 * c:
                        # exact triangular mask on the 128-wide diag square
                        nc.vector.tensor_tensor(
                            pm[:, toff:toff + 128], pm[:, toff:toff + 128],
                            tri[:], op=OP.mult)
                    pms[bi] = pm

                def acc_block(bi):
                    sb_i, toff, is_stop = order[bi]
                    pm = pms.pop(bi)
                    nc.tensor.matmul(sums_ps[:, toff:], onec[:], pm[:, toff:],
                                     start=(bi == 0), stop=is_stop,
                                     skip_group_check=skip_chk)
                    nc.tensor.matmul(yT_ps[:, toff:],
                                     vtile[:, sb_i, h * D:(h + 1) * D],
                                     pm[:, toff:], start=(bi == 0),
                                     stop=is_stop,
                                     skip_group_check=skip_chk)

                nsb = len(order)
                for bi in range(nsb + LAG):
                    if bi < nsb:
                        score_block(bi)
                    if bi >= LAG:
                        acc_block(bi - LAG)
                if pending is not None:
                    finish_head(*pending)
                pending = (h, sums_ps, yT_ps)
            finish_head(*pending)

            # ---- Wo partial + 0.25*x, straight to AR bounce ----
            # h-outer so one yTf weight-load serves 4 output-tile matmuls
            for tt in range(4):
                tsl = slice(tt * 128, (tt + 1) * 128)
                lrows = slice(tt * 128, (tt + 1) * 128)
                wo_pss = []
                for ot in range(4):
                    wo_pss.append(psp.tile([128, 512], f32,
                                           name=f"wops{c}_{tt}_{ot}",
                                           tag="ps", bufs=8))
                for h in range(HL):
                    for ot in range(4):
                        osl = slice(ot * 512, (ot + 1) * 512)
                        nc.tensor.matmul(wo_pss[ot][:], yTfs[h][:, tsl],
                                         wo_r[:, h, osl], start=(h == 0),
                                         stop=(h == HL - 1))
                for ot in range(4):
                    osl = slice(ot * 512, (ot + 1) * 512)
                    aro = wk.tile([128, 512], bf, name=f"aro{c}_{tt}_{ot}",
                                  tag="p", bufs=2)
                    nc.vector.tensor_tensor(aro[:], wo_pss[ot][:],
                                            xq_tts[tt][:, osl], op=OP.add)
                    nc.gpsimd.dma_start(cins[c][lrows, osl], aro[:])

            # ---- AllReduce this chunk within the batch group (async) ----
            if NOAR:
                nc.gpsimd.dma_start(couts[c][:, :], cins[c][:, :])
            else:
                nc.gpsimd.collective_compute(
                    "AllReduce", mybir.AluOpType.add, replica_groups=groups,
                    ins=[cins[c][:, :].opt()], outs=[couts[c][:, :].opt()])
            if c == 2:
                # prefetch the first MLP x1 transpose during attention
                load_x1t(0)

        # ======================= MLP phase (512-token quarters) ==========
        load_x1t(1)
        for hf in range(nch):
            t0 = hf * CH
            x1t = x1t_tiles.pop(hf)

            # wm loads first: their ring slots free up as the attention
            # weights (quarter 0) / previous quarter's wm tiles retire.
            wm_ots = []
            for ot in range(4):
                osl = slice(ot * 512, (ot + 1) * 512)
                wm_ot = big.tile([128, FCT, 512], bf, name=f"wm{hf}_{ot}",
                                 tag="wres", bufs=4)
                nc.sync.dma_start(
                    wm_ot[:],
                    io["wmlpT"].rearrange("(a p) n -> p a n", p=128)[:, :, osl])
                wm_ots.append(wm_ot)

            u2s = {}
            for f in range(FCT):
                if f < 4:
                    wfc_f = wfc_res[f]
                else:
                    wfc_f = wk.tile([128, EC, 128], bf, name=f"wfc{hf}_{f}",
                                    tag="we3", bufs=2)
                    nc.sync.dma_start(
                        wfc_f[:],
                        io["wfcT"].rearrange("(a p) n -> p a n", p=128)
                        [:, :, f * 128:(f + 1) * 128])
                u_ps = psp.tile([128, 512], f32, name=f"ups{hf}_{f}", tag="ps",
                                bufs=8)
                for e in range(EC):
                    nc.tensor.matmul(u_ps[:], wfc_f[:, e, :], x1t[:, e, :],
                                     start=(e == 0), stop=(e == EC - 1))
                ur = wk.tile([128, 512], bf, name=f"ur{hf}_{f}", tag="p",
                             bufs=2)
                nc.scalar.activation(ur[:], u_ps[:], AF.Relu)
                u2 = wk.tile([128, 512], bf, name=f"u2{hf}_{f}", tag="b1k",
                             bufs=16)
                nc.vector.tensor_tensor(u2[:], ur[:], ur[:], op=OP.mult)
                u2s[f] = u2
            # prefetch for upcoming quarters: the x1t transpose's expensive
            # DGE descriptor generation runs during this quarter's out phase
            # (sync queue is quiet then), not at the quarter boundary.
            if hf == 0:
                # emitted after the fc relus so the stats' scalar ops do not
                # block the scalar queue ahead of them
                load_x1rows(0)
                x1_stats(0)
            if hf + 2 < nch:
                load_x1t(hf + 2)
            if hf + 1 < nch:
                load_x1rows(hf + 1)
                x1_stats(hf + 1)

            # tl-outer so one u2 weight-load serves 4 output-tile matmuls
            for tl in range(4):
                tsl = slice(tl * 128, (tl + 1) * 128)
                mps = []
                for ot in range(4):
                    mps.append(psp.tile([128, 512], f32,
                                        name=f"mp{hf}_{tl}_{ot}", tag="ps",
                                        bufs=8))
                for f in range(FCT):
                    for ot in range(4):
                        nc.tensor.matmul(mps[ot][:], u2s[f][:, tsl],
                                         wm_ots[ot][:, f, :],
                                         start=(f == 0), stop=(f == FCT - 1))
                for ot in range(4):
                    osl = slice(ot * 512, (ot + 1) * 512)
                    o_sb = wk.tile([128, 512], bf, name=f"o{hf}_{ot}_{tl}",
                                   tag="tt", bufs=2)
                    # out = mp*4*s2^2 + x1; host scales the 4-core sum by 1/4
                    nc.vector.scalar_tensor_tensor(
                        o_sb[:], mps[ot][:], s2sqs[(hf, tl)][:],
                        x1_tts[(hf, tl)][:, osl], op0=OP.mult, op1=OP.add)
                    rows = slice(t0 + tl * 128, t0 + (tl + 1) * 128)
                    nc.gpsimd.dma_start(io["out"][rows, osl], o_sb[:])

def _pin_act_tables():
    """Force every activation onto natural_log_exp_and_others (it contains
    Exp/Ln/Square/Relu/Copy/Identity) so the table is loaded once instead of
    thrashing between per-function sets. Indices are preserved; the kept
    set's real contents are unchanged, so runtime behavior is sound."""
    import concourse.bacc as bacc_mod
    import concourse.mybir as mybir
    if getattr(bacc_mod, "_act_tables_pinned", False):
        return
    AF = mybir.ActivationFunctionType
    mine = {AF.Exp, AF.Ln, AF.Square, AF.Relu, AF.Copy, AF.Identity}
    orig = bacc_mod.get_activation_tables

    def patched(arch):
        t = orig(arch)
        out = {}
        for name, funcs in t.items():
            if name == "natural_log_exp_and_others":
                out[name] = set(funcs)
            else:
                out[name] = set(funcs) - mine
        return out

    bacc_mod.get_activation_tables = patched
    bacc_mod._act_tables_pinned = True


def build_nc(T=T_FULL, num_devices=NCORES):
    from concourse import bacc
    import concourse.tile as tile
    _pin_act_tables()
    nc = bacc.Bacc("TRN2", target_bir_lowering=False, debug=False,
                   enable_asserts=True, num_devices=num_devices)
    io = declare_io(nc, T)
    with tile.TileContext(nc) as tc:
        emit(tc, io, T)
    nc.compile()
    return nc


def combine_outputs(results, T=T_FULL):
    out = np.zeros((B, T, E), np.float32)
    for c in range(NCORES):
        out[c // 4] += np.asarray(results[c]["out"]).astype(np.float32)
    return 0.25 * out


def kernel(**inputs):
    from concourse.bass_utils import run_bass_kernel_spmd
    in_maps = shard_inputs(**inputs)
    nc = build_nc(T_FULL)
    res = run_bass_kernel_spmd(nc, in_maps, core_ids=list(range(NCORES)))
    return combine_outputs(res.results, T_FULL)

